# revision 8
# baseline (speedup 1.0000x reference)
"""Distributed Trainium2 kernel for fused multi-head attention
(QKV projection + RoPE + softmax attention + output projection).

Problem: x[2,2048,1024], Wqkv[1024,3072], bqkv[3072], Wproj[1024,1024], bproj[1024]
NUM_HEADS=16, head_dim=64, non-causal, RoPE (half-split), scale hd^-0.5.

Sharding over 8 NeuronCores: 2-way batch x 4-way head-group tensor parallel.
Core c: batch b=c//4, head group g=c%4 (heads 4g..4g+3).
Per core: QKV projection for its 4 heads (f32r matmuls), RoPE on DVE in a
stacked half-channel layout, S^T-layout attention with exp on ScalarE
(scale folded, no max subtraction -- scores are ~N(0,1)), PV matmul with a
ones-augmented V producing the softmax denominator for free, AllToAll within
each batch group to exchange head-blocks for token-blocks, then a
token-sharded output projection. Host only shards/transposes/concats.
"""
import sys

sys.path.insert(0, "/opt/trn_rl_repo")

import numpy as np
import ml_dtypes

BF16NP = ml_dtypes.bfloat16

N_CORES = 8
B, S, D = 2, 2048, 1024
H, HD = 16, 64
HPG = 4            # heads per group
TOK = S            # tokens per batch
KT = D // 128      # 8 contraction tiles for D
SK = S // 128      # 16 key tiles
SQC = 2            # sequence chunks of 1024 queries
ROPE_BASE = 10000.0

TRACE = False
LAST_EXEC_NS = None

_CACHE = {}


def _build_nc():
    import concourse.bass as bass  # noqa
    import concourse.bacc as bacc
    import concourse.mybir as mybir
    from concourse import tile

    F32 = mybir.dt.float32
    F32R = mybir.dt.float32r
    BF16 = mybir.dt.bfloat16
    AF = mybir.ActivationFunctionType
    ALU = mybir.AluOpType

    nc = bacc.Bacc("TRN2", target_bir_lowering=False, debug=False,
                   num_devices=N_CORES)

    # ---- per-core DRAM parameters ----
    xT_d = nc.dram_tensor("xT", [D, TOK], F32R, kind="ExternalInput")
    wqa_d = nc.dram_tensor("wqa", [D, 128], F32R, kind="ExternalInput")
    wqb_d = nc.dram_tensor("wqb", [D, 128], F32R, kind="ExternalInput")
    wka_d = nc.dram_tensor("wka", [D, 128], F32R, kind="ExternalInput")
    wkb_d = nc.dram_tensor("wkb", [D, 128], F32R, kind="ExternalInput")
    wv_d = nc.dram_tensor("wv", [D + 1, HPG * 65], F32R, kind="ExternalInput")
    cos_d = nc.dram_tensor("cosT", [128, TOK], F32R, kind="ExternalInput")
    sin_d = nc.dram_tensor("sinT", [128, TOK], F32R, kind="ExternalInput")
    bqa_d = nc.dram_tensor("bqa", [128, 1], F32, kind="ExternalInput")
    bqb_d = nc.dram_tensor("bqb", [128, 1], F32, kind="ExternalInput")
    bka_d = nc.dram_tensor("bka", [128, 1], F32, kind="ExternalInput")
    bkb_d = nc.dram_tensor("bkb", [128, 1], F32, kind="ExternalInput")
    ones_r_d = nc.dram_tensor("ones_r", [1, 128], F32R, kind="ExternalInput")
    ones_b_d = nc.dram_tensor("ones_b", [1, 128], BF16, kind="ExternalInput")
    wp_d = nc.dram_tensor("wp", [D, D], BF16, kind="ExternalInput")
    bp_d = nc.dram_tensor("bp", [1, D], BF16, kind="ExternalInput")
    out_d = nc.dram_tensor("out", [512, D], F32, kind="ExternalOutput")

    with tile.TileContext(nc) as tc:
        with tc.tile_pool(name="const", bufs=1) as constp, \
             tc.tile_pool(name="persist", bufs=1) as persist, \
             tc.tile_pool(name="dram", bufs=1, space="DRAM") as dram:

            # constants / persistent tensors for the whole kernel
            ones_r = constp.tile([1, 128], F32R)
            nc.sync.dma_start(ones_r[:], ones_r_d[:])
            ones_b = constp.tile([1, 128], BF16)
            nc.sync.dma_start(ones_b[:], ones_b_d[:])

            # RoPE'd per-head-pair q/k tiles: pair p holds local heads 2p,2p+1
            qt = [persist.tile([128, TOK], F32R, name=f"qt{p}") for p in range(2)]
            kt_ = [persist.tile([128, TOK], F32R, name=f"ktp{p}") for p in range(2)]
            # V (ones-augmented): sk-block at cols sk*260, head h at +h*65
            vaug = persist.tile([128, SK * HPG * 65], BF16)
            # local attention output, per sq-chunk: tok-tile t at cols t*256
            oloc = [persist.tile([128, 8 * 256], BF16, name=f"oloc{c}")
                    for c in range(SQC)]

            # collective buffers
            a2a_in = [dram.tile([1024, 256], BF16, name=f"a2a_in{c}")
                      for c in range(SQC)]
            a2a_out = [dram.tile([1024, 256], BF16, name=f"a2a_out{c}")
                       for c in range(SQC)]

            # ---------------- phase 1: QKV projection -------------------
            with tc.tile_pool(name="raw", bufs=1) as rawp:
                # stacked raw projections (rows j*32+c = head 4g+j, chan c)
                raw = {nm: rawp.tile([128, TOK], F32R, name=f"raw_{nm}")
                       for nm in ("qa", "qb", "ka", "kb")}
                cosT = rawp.tile([128, TOK], F32R)
                nc.sync.dma_start(cosT[:], cos_d[:])
                sinT = rawp.tile([128, TOK], F32R)
                nc.sync.dma_start(sinT[:], sin_d[:])

                with tc.tile_pool(name="xw", bufs=1) as xw, \
                     tc.tile_pool(name="qk_ps", bufs=3, space="PSUM") as qk_ps, \
                     tc.tile_pool(name="v_ps", bufs=2, space="PSUM") as v_ps:

                    xt = [xw.tile([128, TOK], F32R, name=f"xt{k}")
                          for k in range(KT)]
                    for k in range(KT):
                        nc.sync.dma_start(xt[k][:], xT_d[k * 128:(k + 1) * 128, :])
                    w_sb = {}
                    for nm, dref in (("qa", wqa_d), ("qb", wqb_d),
                                     ("ka", wka_d), ("kb", wkb_d)):
                        t = xw.tile([128, KT * 128], F32R, name=f"w_{nm}")
                        nc.sync.dma_start(
                            t[:].rearrange("p (k n) -> p k n", k=KT),
                            dref[:].rearrange("(k p) n -> p k n", p=128))
                        w_sb[nm] = t
                    wv_sb = xw.tile([128, KT * HPG * 65], F32R)
                    nc.sync.dma_start(
                        wv_sb[:].rearrange("p (k n) -> p k n", k=KT),
                        wv_d[0:D, :].rearrange("(k p) n -> p k n", p=128))
                    wv_ones = xw.tile([1, HPG * 65], F32R)
                    nc.sync.dma_start(wv_ones[:], wv_d[D:D + 1, :])

                    bias_sb = {}
                    for nm, dref in (("qa", bqa_d), ("qb", bqb_d),
                                     ("ka", bka_d), ("kb", bkb_d)):
                        t = constp.tile([128, 1], F32, name=f"b_{nm}")
                        nc.sync.dma_start(t[:], dref[:])
                        bias_sb[nm] = t

                    for nm in ("qa", "qb", "ka", "kb"):
                        for ch in range(4):          # token chunks of 512
                            ps = qk_ps.tile([128, 512], F32, name="qkps",
                                            tag="qkps")
                            for k in range(KT):
                                nc.tensor.matmul(
                                    ps[:],
                                    w_sb[nm][:, k * 128:(k + 1) * 128],
                                    xt[k][:, ch * 512:(ch + 1) * 512],
                                    start=(k == 0), stop=(k == KT - 1))
                            nc.vector.tensor_scalar_add(
                                raw[nm][:, ch * 512:(ch + 1) * 512], ps[:],
                                bias_sb[nm][:])

                    # V: natural layout, ones column via indicator row
                    for sk in range(SK):
                        ps = v_ps.tile([128, HPG * 65], F32, name="vps",
                                       tag="vps")
                        for k in range(KT):
                            nc.tensor.matmul(
                                ps[:],
                                xt[k][:, sk * 128:(sk + 1) * 128],
                                wv_sb[:, k * (HPG * 65):(k + 1) * (HPG * 65)],
                                start=(k == 0), stop=False)
                        nc.tensor.matmul(ps[:], ones_r[:], wv_ones[:],
                                         start=False, stop=True)
                        nc.vector.tensor_copy(
                            vaug[:, sk * (HPG * 65):(sk + 1) * (HPG * 65)],
                            ps[:])

                # ------------- phase 2: RoPE + scatter ------------------
                with tc.tile_pool(name="rope", bufs=2) as ropep, \
                     tc.tile_pool(name="roped", bufs=1) as ropedp:
                    for pref in ("q", "k"):
                        a_r, b_r = raw[pref + "a"], raw[pref + "b"]
                        dst = qt if pref == "q" else kt_
                        m1 = ropep.tile([128, TOK], F32R, name="m1", tag="m1")
                        nc.vector.tensor_tensor(m1[:], a_r[:], cosT[:], ALU.mult)
                        m2 = ropep.tile([128, TOK], F32R, name="m2", tag="m2")
                        nc.vector.tensor_tensor(m2[:], b_r[:], sinT[:], ALU.mult)
                        ar = ropedp.tile([128, TOK], F32R, name=f"ar_{pref}",
                                         tag="arr")
                        nc.vector.tensor_tensor(ar[:], m1[:], m2[:], ALU.subtract)
                        m3 = ropep.tile([128, TOK], F32R, name="m3", tag="m1")
                        nc.vector.tensor_tensor(m3[:], b_r[:], cosT[:], ALU.mult)
                        m4 = ropep.tile([128, TOK], F32R, name="m4", tag="m2")
                        nc.vector.tensor_tensor(m4[:], a_r[:], sinT[:], ALU.mult)
                        br = ropedp.tile([128, TOK], F32R, name=f"br_{pref}",
                                         tag="brr")
                        nc.vector.tensor_tensor(br[:], m3[:], m4[:], ALU.add)
                        # scatter halves into per-head-pair layout
                        for j in range(HPG):       # local head j
                            p, q_ = divmod(j, 2)   # pair p, slot q_
                            nc.sync.dma_start(
                                dst[p][q_ * 64:q_ * 64 + 32, :],
                                ar[j * 32:(j + 1) * 32, :])
                            nc.sync.dma_start(
                                dst[p][q_ * 64 + 32:q_ * 64 + 64, :],
                                br[j * 32:(j + 1) * 32, :])

            # ---------------- phase 3: attention ------------------------
            wpp_ctx = tc.tile_pool(name="wppool", bufs=1)
            wpp = wpp_ctx.__enter__()
            wp_sb = wpp.tile([128, KT * D], BF16)
            nc.sync.dma_start(
                wp_sb[:].rearrange("p (k n) -> p k n", k=KT),
                wp_d[:].rearrange("(k p) n -> p k n", p=128))
            bp_sb = wpp.tile([1, D], BF16)
            nc.sync.dma_start(bp_sb[:], bp_d[:])

            with tc.tile_pool(name="st_ps", bufs=2, space="PSUM") as st_ps, \
                 tc.tile_pool(name="o_ps", bufs=4, space="PSUM") as o_ps, \
                 tc.tile_pool(name="esb", bufs=18) as esb, \
                 tc.tile_pool(name="nrm", bufs=4) as nrmp:
                for sqc in range(SQC):
                    for h in range(HPG):
                        p, q_ = divmod(h, 2)
                        ktile = kt_[p]
                        qtile = qt[p]
                        ops = [o_ps.tile([128, 260], F32, name=f"ops{sqc}_{h}_{half}",
                                         tag="ops")
                               for half in range(2)]
                        ests = []
                        for sk in range(SK):
                            st = st_ps.tile([128, 1024], F32, name="st", tag="st")
                            for n in range(2):
                                nc.tensor.matmul(
                                    st[:, n * 512:(n + 1) * 512],
                                    ktile[q_ * 64:(q_ + 1) * 64,
                                          sk * 128:(sk + 1) * 128],
                                    qtile[q_ * 64:(q_ + 1) * 64,
                                          sqc * 1024 + n * 512:
                                          sqc * 1024 + (n + 1) * 512],
                                    start=True, stop=True)
                            est = esb.tile([128, 1024], BF16,
                                           name=f"est{sk}", tag="est")
                            nc.scalar.activation(est[:], st[:], AF.Exp,
                                                 bias=0.0, scale=0.125)
                            ests.append(est)
                        # each sub's accumulation is one contiguous group:
                        # PSUM start=True resets the whole bank's has_written
                        # bits, so groups sharing a bank must not interleave
                        for sub in range(8):
                            for sk in range(SK):
                                nc.tensor.matmul(
                                    ops[sub // 4][:, (sub % 4) * 65:
                                                  (sub % 4) * 65 + 65],
                                    ests[sk][:, sub * 128:(sub + 1) * 128],
                                    vaug[:, sk * (HPG * 65) + h * 65:
                                         sk * (HPG * 65) + h * 65 + 65],
                                    start=(sk == 0), stop=(sk == SK - 1))
                        # normalize: o / denom -> oloc
                        for sub in range(8):
                            po = ops[sub // 4]
                            rec = nrmp.tile([128, 1], F32, name="rec", tag="rec")
                            nc.vector.reciprocal(
                                rec[:], po[:, (sub % 4) * 65 + 64:
                                           (sub % 4) * 65 + 65])
                            nc.vector.tensor_scalar_mul(
                                oloc[sqc][:, sub * 256 + h * 64:
                                          sub * 256 + h * 64 + 64],
                                po[:, (sub % 4) * 65:(sub % 4) * 65 + 64],
                                rec[:])
                    # ship this sq-chunk: o [1024 tok, 256 chan] -> A2A over
                    # all 8 cores (8 shards of 128 tokens; receiver c gets
                    # token-slice c of both batches, all head groups)
                    nc.sync.dma_start(
                        a2a_in[sqc][:].rearrange("(t p) n -> p t n", p=128),
                        oloc[sqc][:].rearrange("p (t n) -> p t n", t=8))
                    nc.gpsimd.collective_compute(
                        "AllToAll", ALU.bypass,
                        replica_groups=[[0, 1, 2, 3, 4, 5, 6, 7]],
                        ins=[a2a_in[sqc].opt()], outs=[a2a_out[sqc].opt()])

            # ---------------- phase 4: output projection ----------------
            with tc.tile_pool(name="ot", bufs=8) as otp, \
                 tc.tile_pool(name="op_ps", bufs=2, space="PSUM") as op_ps, \
                 tc.tile_pool(name="osb", bufs=2) as osb:
                for sqc in range(SQC):
                    # a2a_out block r (128 rows) = (batch r//4, head grp r%4)
                    # for my 128-token slice. Per batch beta build
                    # oT [1024 chan, 128 tok] via 8 transposing DMAs.
                    for beta in range(2):
                        ot = [otp.tile([128, 128], BF16,
                                       name=f"ot{sqc}_{beta}_{k}", tag="ot")
                              for k in range(KT)]
                        for r4 in range(4):
                            for cc in range(2):
                                nc.sync.dma_start_transpose(
                                    ot[r4 * 2 + cc][:],
                                    a2a_out[sqc][(4 * beta + r4) * 128:
                                                 (4 * beta + r4 + 1) * 128,
                                                 cc * 128:(cc + 1) * 128])
                        for ncol in range(2):
                            ps = op_ps.tile([128, 512], F32, name="oppsum",
                                            tag="oppsum")
                            for k in range(KT):
                                nc.tensor.matmul(
                                    ps[:],
                                    ot[k][:],
                                    wp_sb[:, k * D + ncol * 512:
                                          k * D + (ncol + 1) * 512],
                                    start=(k == 0), stop=False)
                            nc.tensor.matmul(
                                ps[:], ones_b[:],
                                bp_sb[:, ncol * 512:(ncol + 1) * 512],
                                start=False, stop=True)
                            ob = osb.tile([128, 512], F32, name="ob", tag="ob")
                            nc.vector.tensor_copy(ob[:], ps[:])
                            nc.sync.dma_start(
                                out_d[sqc * 256 + beta * 128:
                                      sqc * 256 + (beta + 1) * 128,
                                      ncol * 512:(ncol + 1) * 512], ob[:])
            wpp_ctx.__exit__(None, None, None)
    nc.compile()
    return nc


def _prepare_inputs(x, Wqkv, bqkv, Wproj, bproj):
    """Build the 8 per-core input maps (host-side sharding only)."""
    W3 = Wqkv.reshape(D, 3, H, HD)
    b3 = bqkv.reshape(3, H, HD)

    # RoPE tables, stacked layout [128, TOK]: row j*32+c -> cos(ang[pos, c])
    inv = (1.0 / (ROPE_BASE ** (np.arange(0, HD, 2, dtype=np.float64) / HD)))
    ang = np.arange(TOK, dtype=np.float64)[:, None] * inv[None, :]  # [TOK, 32]
    cosT = np.tile(np.cos(ang).T.astype(np.float32), (4, 1))
    sinT = np.tile(np.sin(ang).T.astype(np.float32), (4, 1))

    wp_bf = Wproj.astype(BF16NP)
    bp_eff = (bqkv[2 * D:3 * D].astype(np.float64) @ Wproj.astype(np.float64)
              + bproj.astype(np.float64)).astype(np.float32)
    bp_bf = bp_eff[None, :].astype(BF16NP)
    ones_r = np.ones((1, 128), np.float32)
    ones_b = np.ones((1, 128), BF16NP)

    in_maps = []
    for c in range(N_CORES):
        b, g = divmod(c, 4)
        hs = slice(4 * g, 4 * g + 4)
        xT = np.ascontiguousarray(x[b].T)  # [D, TOK] fp32

        wqa = np.ascontiguousarray(W3[:, 0, hs, 0:32].reshape(D, 128))
        wqb = np.ascontiguousarray(W3[:, 0, hs, 32:64].reshape(D, 128))
        wka = np.ascontiguousarray(W3[:, 1, hs, 0:32].reshape(D, 128))
        wkb = np.ascontiguousarray(W3[:, 1, hs, 32:64].reshape(D, 128))
        wv = np.zeros((D + 1, HPG * 65), np.float32)
        wv_v = wv[0:D].reshape(D, HPG, 65)
        wv_v[:, :, 0:64] = W3[:, 2, hs, :]
        for j in range(HPG):
            wv[D, j * 65 + 64] = 1.0

        bqa = np.ascontiguousarray(b3[0, hs, 0:32].reshape(128, 1))
        bqb = np.ascontiguousarray(b3[0, hs, 32:64].reshape(128, 1))
        bka = np.ascontiguousarray(b3[1, hs, 0:32].reshape(128, 1))
        bkb = np.ascontiguousarray(b3[1, hs, 32:64].reshape(128, 1))

        in_maps.append({
            "xT": xT, "wqa": wqa, "wqb": wqb, "wka": wka, "wkb": wkb,
            "wv": wv, "cosT": cosT, "sinT": sinT,
            "bqa": bqa, "bqb": bqb, "bka": bka, "bkb": bkb,
            "ones_r": ones_r, "ones_b": ones_b,
            "wp": wp_bf, "bp": bp_bf,
        })
    return in_maps


def kernel(x, Wqkv, bqkv, Wproj, bproj):
    global LAST_EXEC_NS
    from concourse.bass_utils import run_bass_kernel_spmd

    if "nc" not in _CACHE:
        _CACHE["nc"] = _build_nc()
    nc = _CACHE["nc"]

    in_maps = _prepare_inputs(
        np.asarray(x, np.float32), np.asarray(Wqkv, np.float32),
        np.asarray(bqkv, np.float32), np.asarray(Wproj, np.float32),
        np.asarray(bproj, np.float32))

    kw = {}
    if TRACE:
        kw["trace"] = True
    res = run_bass_kernel_spmd(nc, in_maps, core_ids=list(range(N_CORES)), **kw)
    LAST_EXEC_NS = res.exec_time_ns

    out = np.empty((B, S, D), np.float32)
    for c in range(N_CORES):
        r = res.results[c]["out"]
        # core c holds token-slice c (128 tokens) of each sq-chunk, both
        # batches: rows sqc*256 + beta*128 + t -> out[beta, sqc*1024 + c*128+t]
        for sqc in range(SQC):
            for beta in range(B):
                out[beta, sqc * 1024 + c * 128:sqc * 1024 + (c + 1) * 128] = \
                    r[sqc * 256 + beta * 128:sqc * 256 + (beta + 1) * 128]
    return out


# revision 9
# speedup vs baseline: 1.4019x; 1.4019x over previous
"""Distributed Trainium2 kernel for fused multi-head attention
(QKV projection + RoPE + softmax attention + output projection).

Problem: x[2,2048,1024], Wqkv[1024,3072], bqkv[3072], Wproj[1024,1024], bproj[1024]
NUM_HEADS=16, head_dim=64, non-causal, RoPE (half-split), scale hd^-0.5.

Sharding over 8 NeuronCores: 2-way batch x 4-way head-group tensor parallel.
Core c: batch b=c//4, head group g=c%4 (heads 4g..4g+3).
Per core: QKV projection for its 4 heads (bf16 matmuls, fp32 accumulate),
RoPE on DVE in a stacked half-channel layout interleaved with the QKV
matmuls, S^T-layout attention with exp on ScalarE (scale folded, no max
subtraction -- scores are ~N(0,1)), PV matmul with a ones-augmented V
producing the softmax denominator for free, AllToAll over all 8 cores to
exchange head-blocks for token-blocks, then a token-sharded output
projection. Host only shards/transposes/concats.
"""
import sys

sys.path.insert(0, "/opt/trn_rl_repo")

import numpy as np
import ml_dtypes

BF16NP = ml_dtypes.bfloat16

N_CORES = 8
B, S, D = 2, 2048, 1024
H, HD = 16, 64
HPG = 4            # heads per group
TOK = S            # tokens per batch
KT = D // 128      # 8 contraction tiles for D
SK = S // 128      # 16 key tiles
SQC = 2            # sequence chunks of 1024 queries
ROPE_BASE = 10000.0

TRACE = False
LAST_EXEC_NS = None

_CACHE = {}


def _build_nc():
    import concourse.bass as bass  # noqa
    import concourse.bacc as bacc
    import concourse.mybir as mybir
    from concourse import tile

    F32 = mybir.dt.float32
    BF16 = mybir.dt.bfloat16
    AF = mybir.ActivationFunctionType
    ALU = mybir.AluOpType

    nc = bacc.Bacc("TRN2", target_bir_lowering=False, debug=False,
                   num_devices=N_CORES)

    # ---- per-core DRAM parameters (bf16 activations/weights) ----
    xT_d = nc.dram_tensor("xT", [D, TOK], BF16, kind="ExternalInput")
    wqa_d = nc.dram_tensor("wqa", [D, 128], BF16, kind="ExternalInput")
    wqb_d = nc.dram_tensor("wqb", [D, 128], BF16, kind="ExternalInput")
    wka_d = nc.dram_tensor("wka", [D, 128], BF16, kind="ExternalInput")
    wkb_d = nc.dram_tensor("wkb", [D, 128], BF16, kind="ExternalInput")
    wv_d = nc.dram_tensor("wv", [D + 1, HPG * 65], BF16, kind="ExternalInput")
    cos_d = nc.dram_tensor("cosT", [128, TOK], BF16, kind="ExternalInput")
    sin_d = nc.dram_tensor("sinT", [128, TOK], BF16, kind="ExternalInput")
    bqa_d = nc.dram_tensor("bqa", [128, 1], F32, kind="ExternalInput")
    bqb_d = nc.dram_tensor("bqb", [128, 1], F32, kind="ExternalInput")
    bka_d = nc.dram_tensor("bka", [128, 1], F32, kind="ExternalInput")
    bkb_d = nc.dram_tensor("bkb", [128, 1], F32, kind="ExternalInput")
    ones_b_d = nc.dram_tensor("ones_b", [1, 128], BF16, kind="ExternalInput")
    ident_d = nc.dram_tensor("ident", [128, 128], BF16, kind="ExternalInput")
    wp_d = nc.dram_tensor("wp", [D, D], BF16, kind="ExternalInput")
    bp_d = nc.dram_tensor("bp", [1, D], BF16, kind="ExternalInput")
    out_d = nc.dram_tensor("out", [512, D], F32, kind="ExternalOutput")

    with tile.TileContext(nc) as tc:
        with tc.tile_pool(name="const", bufs=1) as constp, \
             tc.tile_pool(name="persist", bufs=1) as persist, \
             tc.tile_pool(name="dram", bufs=1, space="DRAM") as dram:

            ones_b = constp.tile([1, 128], BF16)
            nc.sync.dma_start(ones_b[:], ones_b_d[:])
            ident = constp.tile([128, 128], BF16)
            nc.sync.dma_start(ident[:], ident_d[:])
            bias_sb = {}
            for nm, dref in (("qa", bqa_d), ("qb", bqb_d),
                             ("ka", bka_d), ("kb", bkb_d)):
                t = constp.tile([128, 1], F32, name=f"b_{nm}")
                nc.sync.dma_start(t[:], dref[:])
                bias_sb[nm] = t

            # RoPE'd per-head-pair q/k tiles: pair p holds local heads 2p,2p+1
            qt = [persist.tile([128, TOK], BF16, name=f"qt{p}") for p in range(2)]
            kt_ = [persist.tile([128, TOK], BF16, name=f"ktp{p}") for p in range(2)]
            # V (ones-augmented): sk-block at cols sk*260, head h at +h*65
            vaug = persist.tile([128, SK * HPG * 65], BF16)
            # local attention output, per sq-chunk: tok-tile t at cols t*256
            oloc = [persist.tile([128, 8 * 256], BF16, name=f"oloc{c}")
                    for c in range(SQC)]

            a2a_in = [dram.tile([1024, 256], BF16, name=f"a2a_in{c}")
                      for c in range(SQC)]
            a2a_out = [dram.tile([1024, 256], BF16, name=f"a2a_out{c}")
                       for c in range(SQC)]

            # ---------------- phase 1+2: QKV projection + RoPE ----------
            with tc.tile_pool(name="raw", bufs=1) as rawp:
                raw = {nm: rawp.tile([128, TOK], BF16, name=f"raw_{nm}")
                       for nm in ("qa", "qb", "ka", "kb")}
                cosT = rawp.tile([128, TOK], BF16)
                nc.sync.dma_start(cosT[:], cos_d[:])
                sinT = rawp.tile([128, TOK], BF16)
                nc.sync.dma_start(sinT[:], sin_d[:])

                with tc.tile_pool(name="xw", bufs=1) as xw, \
                     tc.tile_pool(name="qk_ps", bufs=3, space="PSUM") as qk_ps, \
                     tc.tile_pool(name="v_ps", bufs=2, space="PSUM") as v_ps, \
                     tc.tile_pool(name="rope", bufs=2) as ropep:

                    # weights first (small), then x split across queues
                    w_sb = {}
                    for nm, dref in (("qa", wqa_d), ("qb", wqb_d),
                                     ("ka", wka_d), ("kb", wkb_d)):
                        t = xw.tile([128, KT * 128], BF16, name=f"w_{nm}")
                        nc.sync.dma_start(
                            t[:].rearrange("p (k n) -> p k n", k=KT),
                            dref[:].rearrange("(k p) n -> p k n", p=128))
                        w_sb[nm] = t
                    wv_sb = xw.tile([128, KT * HPG * 65], BF16)
                    for half in range(2):
                        nc.sync.dma_start(
                            wv_sb[:, half * 4 * HPG * 65:
                                  (half + 1) * 4 * HPG * 65].rearrange(
                                      "p (k n) -> p k n", k=4),
                            wv_d[half * 512:(half + 1) * 512, :].rearrange(
                                "(k p) n -> p k n", p=128))
                    wv_ones = xw.tile([1, HPG * 65], BF16)
                    nc.sync.dma_start(wv_ones[:], wv_d[D:D + 1, :])

                    xt = [xw.tile([128, TOK], BF16, name=f"xt{k}")
                          for k in range(KT)]
                    for k in range(KT):
                        for half in range(2):
                            nc.sync.dma_start(
                                xt[k][:, half * 1024:(half + 1) * 1024],
                                xT_d[k * 128:(k + 1) * 128,
                                     half * 1024:(half + 1) * 1024])

                    # token-chunk loop: QKV matmuls, evac+bias, RoPE, scatter
                    for ch in range(4):
                        c0, c1 = ch * 512, (ch + 1) * 512
                        for nm in ("qa", "qb", "ka", "kb"):
                            ps = qk_ps.tile([128, 512], F32, name="qkps",
                                            tag="qkps")
                            for k in range(KT):
                                nc.tensor.matmul(
                                    ps[:],
                                    w_sb[nm][:, k * 128:(k + 1) * 128],
                                    xt[k][:, c0:c1],
                                    start=(k == 0), stop=(k == KT - 1))
                            nc.vector.tensor_scalar_add(
                                raw[nm][:, c0:c1], ps[:], bias_sb[nm][:])
                        # RoPE for this chunk (stacked layout, full-lane DVE)
                        for pref in ("q", "k"):
                            a_r, b_r = raw[pref + "a"], raw[pref + "b"]
                            dst = qt if pref == "q" else kt_
                            m1 = ropep.tile([128, 512], BF16, name="m1", tag="m1")
                            nc.vector.tensor_tensor(
                                m1[:], a_r[:, c0:c1], cosT[:, c0:c1], ALU.mult)
                            m2 = ropep.tile([128, 512], BF16, name="m2", tag="m2")
                            nc.vector.tensor_tensor(
                                m2[:], b_r[:, c0:c1], sinT[:, c0:c1], ALU.mult)
                            ar = ropep.tile([128, 512], BF16, name="ar", tag="ar")
                            nc.vector.tensor_tensor(ar[:], m1[:], m2[:],
                                                    ALU.subtract)
                            m3 = ropep.tile([128, 512], BF16, name="m3", tag="m1")
                            nc.vector.tensor_tensor(
                                m3[:], b_r[:, c0:c1], cosT[:, c0:c1], ALU.mult)
                            m4 = ropep.tile([128, 512], BF16, name="m4", tag="m2")
                            nc.vector.tensor_tensor(
                                m4[:], a_r[:, c0:c1], sinT[:, c0:c1], ALU.mult)
                            br = ropep.tile([128, 512], BF16, name="br", tag="br")
                            nc.vector.tensor_tensor(br[:], m3[:], m4[:], ALU.add)
                            for j in range(HPG):       # local head j
                                p, q_ = divmod(j, 2)   # pair p, slot q_
                                nc.sync.dma_start(
                                    dst[p][q_ * 64:q_ * 64 + 32, c0:c1],
                                    ar[j * 32:(j + 1) * 32, :])
                                nc.sync.dma_start(
                                    dst[p][q_ * 64 + 32:q_ * 64 + 64, c0:c1],
                                    br[j * 32:(j + 1) * 32, :])

                    # V: natural layout, ones column via indicator row
                    # (emitted after q/k so PE covers the RoPE tail)
                    for sk in range(SK):
                        ps = v_ps.tile([128, HPG * 65], F32, name="vps",
                                       tag="vps")
                        for k in range(KT):
                            nc.tensor.matmul(
                                ps[:],
                                xt[k][:, sk * 128:(sk + 1) * 128],
                                wv_sb[:, k * (HPG * 65):(k + 1) * (HPG * 65)],
                                start=(k == 0), stop=False)
                        nc.tensor.matmul(ps[:], ones_b[:], wv_ones[:],
                                         start=False, stop=True)
                        nc.vector.tensor_copy(
                            vaug[:, sk * (HPG * 65):(sk + 1) * (HPG * 65)],
                            ps[:])

            # ---------------- phase 3: attention ------------------------
            wpp_ctx = tc.tile_pool(name="wppool", bufs=1)
            wpp = wpp_ctx.__enter__()
            wp_sb = wpp.tile([128, KT * D], BF16)
            for quart in range(4):
                nc.sync.dma_start(
                    wp_sb[:, quart * 2 * D:(quart + 1) * 2 * D].rearrange(
                        "p (k n) -> p k n", k=2),
                    wp_d[quart * 256:(quart + 1) * 256, :].rearrange(
                        "(k p) n -> p k n", p=128))
            bp_sb = wpp.tile([1, D], BF16)
            nc.sync.dma_start(bp_sb[:], bp_d[:])

            with tc.tile_pool(name="st_ps", bufs=3, space="PSUM") as st_ps, \
                 tc.tile_pool(name="o_ps", bufs=2, space="PSUM") as o_ps, \
                 tc.tile_pool(name="esb", bufs=18) as esb, \
                 tc.tile_pool(name="nrm", bufs=4) as nrmp:
                for sqc in range(SQC):
                    for h in range(HPG):
                        p, q_ = divmod(h, 2)
                        ktile = kt_[p]
                        qtile = qt[p]
                        ops = [o_ps.tile([128, 260], F32,
                                         name=f"ops{sqc}_{h}_{half}", tag="ops")
                               for half in range(2)]
                        ests = []
                        for sk in range(SK):
                            st = st_ps.tile([128, 1024], F32, name="st", tag="st")
                            for n in range(2):
                                nc.tensor.matmul(
                                    st[:, n * 512:(n + 1) * 512],
                                    ktile[q_ * 64:(q_ + 1) * 64,
                                          sk * 128:(sk + 1) * 128],
                                    qtile[q_ * 64:(q_ + 1) * 64,
                                          sqc * 1024 + n * 512:
                                          sqc * 1024 + (n + 1) * 512],
                                    start=True, stop=True)
                            est = esb.tile([128, 1024], BF16,
                                           name=f"est{sk}", tag="est")
                            nc.scalar.activation(est[:], st[:], AF.Exp,
                                                 bias=0.0, scale=0.125)
                            ests.append(est)
                        # each sub's accumulation is one contiguous group:
                        # PSUM start=True resets the whole bank's has_written
                        # bits, so groups sharing a bank must not interleave
                        for sub in range(8):
                            for sk in range(SK):
                                nc.tensor.matmul(
                                    ops[sub // 4][:, (sub % 4) * 65:
                                                  (sub % 4) * 65 + 65],
                                    ests[sk][:, sub * 128:(sub + 1) * 128],
                                    vaug[:, sk * (HPG * 65) + h * 65:
                                         sk * (HPG * 65) + h * 65 + 65],
                                    start=(sk == 0), stop=(sk == SK - 1))
                        # normalize: o / denom -> oloc
                        for sub in range(8):
                            po = ops[sub // 4]
                            rec = nrmp.tile([128, 1], F32, name="rec", tag="rec")
                            nc.vector.reciprocal(
                                rec[:], po[:, (sub % 4) * 65 + 64:
                                           (sub % 4) * 65 + 65])
                            nc.vector.tensor_scalar_mul(
                                oloc[sqc][:, sub * 256 + h * 64:
                                          sub * 256 + h * 64 + 64],
                                po[:, (sub % 4) * 65:(sub % 4) * 65 + 64],
                                rec[:])
                        # ship this head's slice to the A2A bounce right away
                        nc.sync.dma_start(
                            a2a_in[sqc][:, h * 64:(h + 1) * 64].rearrange(
                                "(t p) n -> p t n", p=128),
                            oloc[sqc][:].rearrange(
                                "p (t n) -> p t n", t=8)[:, :,
                                                         h * 64:(h + 1) * 64])
                    # A2A over all 8 cores: 8 shards of 128 tokens; receiver
                    # c gets token-slice c of both batches, all head groups
                    nc.gpsimd.collective_compute(
                        "AllToAll", ALU.bypass,
                        replica_groups=[[0, 1, 2, 3, 4, 5, 6, 7]],
                        ins=[a2a_in[sqc].opt()], outs=[a2a_out[sqc].opt()])

            # ---------------- phase 4: output projection ----------------
            with tc.tile_pool(name="ot", bufs=16) as otp, \
                 tc.tile_pool(name="otin", bufs=16) as otinp, \
                 tc.tile_pool(name="tr_ps", bufs=4, space="PSUM") as tr_ps, \
                 tc.tile_pool(name="op_ps", bufs=2, space="PSUM") as op_ps, \
                 tc.tile_pool(name="osb", bufs=2) as osb:
                for sqc in range(SQC):
                    # a2a_out block r (128 rows) = (batch r//4, head grp r%4)
                    # for my 128-token slice. Per batch beta build
                    # oT [1024 chan, 128 tok] with PE transposes (o @ I).
                    for beta in range(2):
                        ot = []
                        for k in range(KT):
                            r4, cc = divmod(k, 2)
                            oin = otinp.tile([128, 128], BF16,
                                             name=f"oin{sqc}_{beta}_{k}",
                                             tag="oin")
                            nc.sync.dma_start(
                                oin[:],
                                a2a_out[sqc][(4 * beta + r4) * 128:
                                             (4 * beta + r4 + 1) * 128,
                                             cc * 128:(cc + 1) * 128])
                            tp = tr_ps.tile([128, 128], F32, name="tp", tag="tp")
                            nc.tensor.matmul(tp[:], oin[:], ident[:],
                                             start=True, stop=True)
                            o_t = otp.tile([128, 128], BF16,
                                           name=f"ot{sqc}_{beta}_{k}", tag="ot")
                            nc.vector.tensor_copy(o_t[:], tp[:])
                            ot.append(o_t)
                        for ncol in range(2):
                            ps = op_ps.tile([128, 512], F32, name="oppsum",
                                            tag="oppsum")
                            for k in range(KT):
                                nc.tensor.matmul(
                                    ps[:],
                                    ot[k][:],
                                    wp_sb[:, k * D + ncol * 512:
                                          k * D + (ncol + 1) * 512],
                                    start=(k == 0), stop=False)
                            nc.tensor.matmul(
                                ps[:], ones_b[:],
                                bp_sb[:, ncol * 512:(ncol + 1) * 512],
                                start=False, stop=True)
                            ob = osb.tile([128, 512], F32, name="ob", tag="ob")
                            nc.vector.tensor_copy(ob[:], ps[:])
                            nc.sync.dma_start(
                                out_d[sqc * 256 + beta * 128:
                                      sqc * 256 + (beta + 1) * 128,
                                      ncol * 512:(ncol + 1) * 512], ob[:])
            wpp_ctx.__exit__(None, None, None)
    nc.compile()
    return nc


def _prepare_inputs(x, Wqkv, bqkv, Wproj, bproj):
    """Build the 8 per-core input maps (host-side sharding only)."""
    W3 = Wqkv.reshape(D, 3, H, HD)
    b3 = bqkv.reshape(3, H, HD)

    # RoPE tables, stacked layout [128, TOK]: row j*32+c -> cos(ang[pos, c])
    inv = (1.0 / (ROPE_BASE ** (np.arange(0, HD, 2, dtype=np.float64) / HD)))
    ang = np.arange(TOK, dtype=np.float64)[:, None] * inv[None, :]  # [TOK, 32]
    cosT = np.tile(np.cos(ang).T.astype(np.float32), (4, 1)).astype(BF16NP)
    sinT = np.tile(np.sin(ang).T.astype(np.float32), (4, 1)).astype(BF16NP)

    wp_bf = Wproj.astype(BF16NP)
    bp_eff = (bqkv[2 * D:3 * D].astype(np.float64) @ Wproj.astype(np.float64)
              + bproj.astype(np.float64)).astype(np.float32)
    bp_bf = bp_eff[None, :].astype(BF16NP)
    ones_b = np.ones((1, 128), BF16NP)
    ident = np.eye(128, dtype=np.float32).astype(BF16NP)

    in_maps = []
    for c in range(N_CORES):
        b, g = divmod(c, 4)
        hs = slice(4 * g, 4 * g + 4)
        xT = np.ascontiguousarray(x[b].T).astype(BF16NP)  # [D, TOK]

        wqa = np.ascontiguousarray(W3[:, 0, hs, 0:32].reshape(D, 128)).astype(BF16NP)
        wqb = np.ascontiguousarray(W3[:, 0, hs, 32:64].reshape(D, 128)).astype(BF16NP)
        wka = np.ascontiguousarray(W3[:, 1, hs, 0:32].reshape(D, 128)).astype(BF16NP)
        wkb = np.ascontiguousarray(W3[:, 1, hs, 32:64].reshape(D, 128)).astype(BF16NP)
        wv = np.zeros((D + 1, HPG * 65), np.float32)
        wv_v = wv[0:D].reshape(D, HPG, 65)
        wv_v[:, :, 0:64] = W3[:, 2, hs, :]
        for j in range(HPG):
            wv[D, j * 65 + 64] = 1.0
        wv = wv.astype(BF16NP)

        bqa = np.ascontiguousarray(b3[0, hs, 0:32].reshape(128, 1))
        bqb = np.ascontiguousarray(b3[0, hs, 32:64].reshape(128, 1))
        bka = np.ascontiguousarray(b3[1, hs, 0:32].reshape(128, 1))
        bkb = np.ascontiguousarray(b3[1, hs, 32:64].reshape(128, 1))

        in_maps.append({
            "xT": xT, "wqa": wqa, "wqb": wqb, "wka": wka, "wkb": wkb,
            "wv": wv, "cosT": cosT, "sinT": sinT,
            "bqa": bqa, "bqb": bqb, "bka": bka, "bkb": bkb,
            "ones_b": ones_b, "ident": ident,
            "wp": wp_bf, "bp": bp_bf,
        })
    return in_maps


def kernel(x, Wqkv, bqkv, Wproj, bproj):
    global LAST_EXEC_NS
    from concourse.bass_utils import run_bass_kernel_spmd

    if "nc" not in _CACHE:
        _CACHE["nc"] = _build_nc()
    nc = _CACHE["nc"]

    in_maps = _prepare_inputs(
        np.asarray(x, np.float32), np.asarray(Wqkv, np.float32),
        np.asarray(bqkv, np.float32), np.asarray(Wproj, np.float32),
        np.asarray(bproj, np.float32))

    kw = {}
    if TRACE:
        kw["trace"] = True
    res = run_bass_kernel_spmd(nc, in_maps, core_ids=list(range(N_CORES)), **kw)
    LAST_EXEC_NS = res.exec_time_ns

    out = np.empty((B, S, D), np.float32)
    for c in range(N_CORES):
        r = res.results[c]["out"]
        # core c holds token-slice c (128 tokens) of each sq-chunk, both
        # batches: rows sqc*256 + beta*128 + t -> out[beta, sqc*1024 + c*128+t]
        for sqc in range(SQC):
            for beta in range(B):
                out[beta, sqc * 1024 + c * 128:sqc * 1024 + (c + 1) * 128] = \
                    r[sqc * 256 + beta * 128:sqc * 256 + (beta + 1) * 128]
    return out
